# revision 19
# baseline (speedup 1.0000x reference)
"""Trainium2 Bass kernel for nn_Algin (deformable-conv alignment).

kernel(**inputs) -> (a0, a1, a2), matching the jax reference.

Sharding: 8 cores = 4 batches x 2 H-halves (SPMD). The s=1 half runs on a
vertically FLIPPED image with ky-flipped conv weights and y-negated offset
weights, so every core executes identical top-aligned geometry (out rows
[0,96) of its flipped level); the host un-flips and assembles.

Deformable sampling uses the exact hat-function identity (valid since
|offset| < 1 here): bilinear over rows = X0 + relu(-t)*(X(-1)-X0)
+ relu(t)*(X(+1)-X0), applied separably in y then x, on a
[144=(dg,kk,cg)] partition layout with kk-pre-shifted replicas of the
zero-padded feature map. No gather needed.
"""
import numpy as np

C = 16
DG = 8
KK = 9
H = 192
Wp = 196                  # 2 + 192 + 2 padded width
WI = [192, 190, 188]      # level widths
HI = [192, 190, 188]      # level heights
RFB = [108, 106, 104]     # f1/f2/f3 band rows (row r <-> global y = r-4)
RO = 96                   # out rows computed per core per level
RB = 16                   # sampling band rows
RBF = 27                  # fe conv band rows

_CACHE = {}


def _bands(total, size):
    out, p = [], 0
    while p < total:
        out.append((p, min(size, total - p)))
        p += size
    return out


def _build_program():
    import concourse.bacc as bacc
    import concourse.mybir as mybir
    from concourse.tile import TileContext

    f32 = mybir.dt.float32
    bf16 = mybir.dt.bfloat16
    RELU = mybir.ActivationFunctionType.Relu
    IDENT = mybir.ActivationFunctionType.Identity

    nc = bacc.Bacc()

    imgU = nc.declare_dram_parameter("imgU", [C, 110, Wp], f32, isOutput=False)
    imgR = nc.declare_dram_parameter("imgR", [C, 110, Wp], f32, isOutput=False)
    wfe_d = [nc.declare_dram_parameter(f"wfe{i}", [48, 3, 16], f32, isOutput=False)
             for i in range(3)]
    woff_d = [nc.declare_dram_parameter(f"woff{i}", [96, 3, 16], f32, isOutput=False)
              for i in range(3)]
    wdY_d = nc.declare_dram_parameter("wdY", [48, 3, 72], f32, isOutput=False)
    wdX_d = nc.declare_dram_parameter("wdX", [48, 3, 72], f32, isOutput=False)
    wtA_d = nc.declare_dram_parameter("wtA", [128, 16], f32, isOutput=False)
    wtB_d = nc.declare_dram_parameter("wtB", [16, 16], f32, isOutput=False)
    bias_d = nc.declare_dram_parameter("biases", [C, 8], f32, isOutput=False)
    db2_d = nc.declare_dram_parameter("dcnb2", [72, 2], f32, isOutput=False)
    outs = [nc.declare_dram_parameter(f"out{i}", [C, RO, WI[i]], f32,
                                      isOutput=True) for i in range(3)]

    fU = [nc.dram_tensor(f"fU{i}", [C, RFB[i], Wp], bf16) for i in range(3)]
    fR = [nc.dram_tensor(f"fR{i}", [C, RFB[i], Wp], bf16) for i in range(3)]
    import os
    _dbg = os.environ.get("KDEBUG", "") == "1"
    fdbg = [nc.declare_dram_parameter(f"fdbg{i}", [C, RFB[i], Wp], f32,
                                      isOutput=True) for i in range(3)] if _dbg else None
    _dbg2 = os.environ.get("KDEBUG2", "") == "1"
    if _dbg2:
        d_oy = nc.declare_dram_parameter("d_oy", [72, RB * Wp], f32, isOutput=True)
        d_g = nc.declare_dram_parameter("d_g", [128, RB * Wp], f32, isOutput=True)
        d_uy = nc.declare_dram_parameter("d_uy", [128, RB * Wp], f32, isOutput=True)

    with TileContext(nc) as tc:
        with tc.tile_pool(name="const", bufs=1) as cpool:
            wfe_t = []
            for i in range(3):
                t = cpool.tile([48, 48], bf16, tag=f"wfe{i}")
                nc.gpsimd.dma_start(out=t[:], in_=wfe_d[i][:])
                wfe_t.append(t)
            woff_t = []
            for i in range(3):
                t = cpool.tile([96, 48], bf16, tag=f"woff{i}")
                nc.gpsimd.dma_start(out=t[:], in_=woff_d[i][:])
                woff_t.append(t)
            wdY_t = cpool.tile([48, 3 * 72], bf16, tag="wdY")
            nc.gpsimd.dma_start(out=wdY_t[:], in_=wdY_d[:])
            wdX_t = cpool.tile([48, 3 * 72], bf16, tag="wdX")
            nc.gpsimd.dma_start(out=wdX_t[:], in_=wdX_d[:])
            wtA_t = cpool.tile([128, 16], bf16, tag="wtA")
            nc.gpsimd.dma_start(out=wtA_t[:], in_=wtA_d[:])
            wtB_t = cpool.tile([16, 16], bf16, tag="wtB")
            nc.gpsimd.dma_start(out=wtB_t[:], in_=wtB_d[:])
            bias_t = cpool.tile([C, 8], f32, tag="bias")
            nc.sync.dma_start(out=bias_t[:], in_=bias_d[:])
            db2_t = cpool.tile([72, 2], f32, tag="db2")
            nc.sync.dma_start(out=db2_t[:], in_=db2_d[:])
            zerob = cpool.tile([C, 1024], bf16, tag="zerob")
            nc.vector.memset(zerob[:], 0.0)

            # ---------------- feature extraction ----------------
            with (
                tc.tile_pool(name="fe", bufs=2) as fepool,
                tc.tile_pool(name="fes", bufs=4) as fespool,
                tc.tile_pool(name="fps", bufs=4, space="PSUM") as fppool,
            ):
                def fe_conv(src, dst, wt, bcol, pad, w_out, zero_top,
                            cast=False):
                    dma_eng = nc.gpsimd if cast else nc.sync
                    rbase = 1 if pad == 1 else 2
                    base = 2
                    r_out = dst.shape[1]
                    for j0, rn in _bands(r_out, RBF):
                        patch = fepool.tile([48, (RBF + 2) * Wp], bf16,
                                            tag="fepatch")
                        pv = patch[:].rearrange("p (r c) -> p r c", c=Wp)
                        for kx in range(3):
                            dma_eng.dma_start(
                                out=pv[kx * 16:(kx + 1) * 16, 0:rn + 2,
                                       0:Wp - kx],
                                in_=src[:, j0:j0 + rn + 2, kx:Wp])
                        for jj in range(0, rn, 2):
                            nr = min(2, rn - jj)
                            ps = fppool.tile([16, 512], f32, tag="fps")
                            for ky in range(3):
                                nc.tensor.matmul(
                                    ps[:, 0:nr * w_out],
                                    wt[:, ky * 16:(ky + 1) * 16],
                                    pv[:, jj + ky:jj + ky + nr,
                                       rbase:rbase + w_out],
                                    start=(ky == 0), stop=(ky == 2))
                            stg = fespool.tile([16, 2 * Wp], bf16, tag="festg")
                            nc.scalar.activation(
                                stg[:, 0:nr * w_out], ps[:, 0:nr * w_out],
                                RELU, bias=bias_t[:, bcol:bcol + 1])
                            nc.sync.dma_start(
                                out=dst[:, j0 + jj:j0 + jj + nr,
                                        base:base + w_out],
                                in_=stg[:, 0:nr * w_out])
                    # zero horizontal pads
                    nc.sync.dma_start(out=dst[:, :, 0:base],
                                      in_=zerob[:, 0:r_out * base])
                    rw = Wp - base - w_out
                    nc.sync.dma_start(out=dst[:, :, base + w_out:Wp],
                                      in_=zerob[:, 0:r_out * rw])
                    if zero_top:
                        nc.sync.dma_start(out=dst[:, 0:4, :],
                                          in_=zerob[:, 0:4 * Wp])

                fe_conv(imgU, fU[0], wfe_t[0], 0, 1, 192, True, cast=True)
                fe_conv(fU[0], fU[1], wfe_t[1], 1, 0, 190, True)
                fe_conv(fU[1], fU[2], wfe_t[2], 2, 0, 188, True)
                if _dbg:
                    for i in range(3):
                        dt_ = fepool.tile([16, RFB[i] * Wp], f32, tag="dbg")
                        nc.sync.dma_start(out=dt_[:], in_=fU[i][:])
                        nc.sync.dma_start(out=fdbg[i][:], in_=dt_[:])
                fe_conv(imgR, fR[0], wfe_t[0], 0, 1, 192, True, cast=True)
                fe_conv(fR[0], fR[1], wfe_t[1], 1, 0, 190, True)
                fe_conv(fR[1], fR[2], wfe_t[2], 2, 0, 188, True)

            # ---------------- levels ----------------
            with (
                tc.tile_pool(name="lvl", bufs=1) as lp,
                tc.tile_pool(name="lvs", bufs=4) as lsp,
                tc.tile_pool(name="lps", bufs=4, space="PSUM") as pp,
            ):
                for lvl in range(3):
                    Wl = WI[lvl]
                    NX = (RB + 4) * Wp
                    for b0, rb in _bands(RO, RB):
                        N = rb * Wp
                        # ---- patch96 for off_feat conv ----
                        p96 = lp.tile([96, (RB + 4) * Wp], bf16, tag="p96")
                        p96v = p96[:].rearrange("p (r c) -> p r c", c=Wp)
                        for kx in range(3):
                            for im, fsrc in ((0, fR[lvl]), (1, fU[lvl])):
                                nc.sync.dma_start(
                                    out=p96v[kx * 32 + im * 16:
                                             kx * 32 + im * 16 + 16,
                                             0:rb + 4, 0:Wp - kx],
                                    in_=fsrc[:, b0 + 2:b0 + rb + 6, kx:Wp])
                        # ---- off_feat conv (pad=1, 32->16) ----
                        offt = lp.tile([16, (RB + 2) * Wp], bf16, tag="offt")
                        otv = offt[:].rearrange("p (r c) -> p r c", c=Wp)
                        for jj in range(0, rb + 2, 2):
                            nr = min(2, rb + 2 - jj)
                            ps = pp.tile([16, 512], f32, tag="ps16")
                            for ky in range(3):
                                nc.tensor.matmul(
                                    ps[:, 0:nr * Wl],
                                    woff_t[lvl][:, ky * 16:(ky + 1) * 16],
                                    p96v[:, jj + ky:jj + ky + nr, 1:1 + Wl],
                                    start=(ky == 0), stop=(ky == 2))
                            nc.scalar.activation(
                                otv[:, jj:jj + nr, 2:2 + Wl],
                                ps[:, 0:nr * Wl].rearrange(
                                    "p (r c) -> p r c", c=Wl),
                                RELU, bias=bias_t[:, 3 + lvl:4 + lvl])
                        # zero pads of off_feat (cols, and top row on band 0)
                        nc.vector.memset(otv[:, :, 0:2], 0.0)
                        nc.vector.memset(otv[:, :, 2 + Wl:Wp], 0.0)
                        if b0 == 0:
                            nc.vector.memset(otv[:, 0:1, :], 0.0)
                        # ---- dcn_off conv (pad=1, 16->72+72) ----
                        p48 = lp.tile([48, (RB + 2) * Wp], bf16, tag="p48")
                        p48v = p48[:].rearrange("p (r c) -> p r c", c=Wp)
                        for kx in range(3):
                            nc.sync.dma_start(
                                out=p48v[kx * 16:(kx + 1) * 16, :,
                                         0:Wp - kx],
                                in_=otv[:, :, kx:Wp])
                        oy72 = lp.tile([72, RB * Wp], bf16, tag="oy72")
                        ox72 = lp.tile([72, RB * Wp], bf16, tag="ox72")
                        oy72v = oy72[:].rearrange("p (r c) -> p r c", c=Wp)
                        ox72v = ox72[:].rearrange("p (r c) -> p r c", c=Wp)
                        for jj in range(0, rb, 2):
                            nr = min(2, rb - jj)
                            psY = pp.tile([72, 512], f32, tag="ps72")
                            psX = pp.tile([72, 512], f32, tag="ps72")
                            for ky in range(3):
                                nc.tensor.matmul(
                                    psY[:, 0:nr * Wl],
                                    wdY_t[:, ky * 72:(ky + 1) * 72],
                                    p48v[:, jj + ky:jj + ky + nr, 1:1 + Wl],
                                    start=(ky == 0), stop=(ky == 2))
                                nc.tensor.matmul(
                                    psX[:, 0:nr * Wl],
                                    wdX_t[:, ky * 72:(ky + 1) * 72],
                                    p48v[:, jj + ky:jj + ky + nr, 1:1 + Wl],
                                    start=(ky == 0), stop=(ky == 2))
                            nc.scalar.activation(
                                oy72v[:, jj:jj + nr, 2:2 + Wl],
                                psY[:, 0:nr * Wl].rearrange(
                                    "p (r c) -> p r c", c=Wl),
                                IDENT, bias=db2_t[:, 0:1])
                            nc.scalar.activation(
                                ox72v[:, jj:jj + nr, 2:2 + Wl],
                                psX[:, 0:nr * Wl].rearrange(
                                    "p (r c) -> p r c", c=Wl),
                                IDENT, bias=db2_t[:, 1:2])
                        # ---- dup offsets to 144 = (dg,kk,cg) ----
                        oyA = lp.tile([128, RB * Wp], bf16, tag="oyA")
                        oxA = lp.tile([128, RB * Wp], bf16, tag="oxA")
                        oyB = lp.tile([16, RB * Wp], bf16, tag="oyB")
                        oxB = lp.tile([16, RB * Wp], bf16, tag="oxB")
                        for cg in range(2):
                            nc.sync.dma_start(
                                out=oyA[:].rearrange("(u v) f -> u v f", v=2)
                                [:, cg, 0:N], in_=oy72[0:64, 0:N])
                            nc.sync.dma_start(
                                out=oxA[:].rearrange("(u v) f -> u v f", v=2)
                                [:, cg, 0:N], in_=ox72[0:64, 0:N])
                            nc.sync.dma_start(
                                out=oyB[:].rearrange("(u v) f -> u v f", v=2)
                                [:, cg, 0:N], in_=oy72[64:72, 0:N])
                            nc.sync.dma_start(
                                out=oxB[:].rearrange("(u v) f -> u v f", v=2)
                                [:, cg, 0:N], in_=ox72[64:72, 0:N])
                        # ---- hat fields ----
                        fld = {}
                        for nm, src_t, sc, npart in (
                                ("uyA", oyA, -1.0, 128), ("vyA", oyA, 1.0, 128),
                                ("uxA", oxA, -1.0, 128), ("vxA", oxA, 1.0, 128),
                                ("uyB", oyB, -1.0, 16), ("vyB", oyB, 1.0, 16),
                                ("uxB", oxB, -1.0, 16), ("vxB", oxB, 1.0, 16)):
                            t = lp.tile([npart, RB * Wp], bf16, tag=nm)
                            nc.scalar.activation(t[:, 0:N], src_t[:, 0:N],
                                                 RELU, scale=sc)
                            fld[nm] = t
                        # ---- X144 replicas ----
                        XA = lp.tile([128, NX], bf16, tag="XA")
                        XB = lp.tile([16, NX], bf16, tag="XB")
                        NXb = (rb + 4) * Wp
                        sU = fU[lvl]
                        for kk in range(KK):
                            ky, kx = kk // 3, kk % 3
                            # src band row = (b0-2+rr) + (ky-1) + 4
                            r0 = b0 + ky + 1
                            c0d = max(0, 1 - kx)
                            c1d = Wp - max(0, kx - 1)
                            dstt = XA if kk < 8 else XB
                            pb = kk * 16 if kk < 8 else 0
                            nc.sync.dma_start(
                                out=dstt[:].rearrange(
                                    "p (r c) -> p r c", c=Wp)
                                [pb:pb + 16, 0:rb + 4, c0d:c1d],
                                in_=sU[:, r0:r0 + rb + 4,
                                       c0d + kx - 1:c1d + kx - 1])
                        # ---- y-lerp at 3 column shifts, then x-combine ----
                        # D1 = Xm-X0, D2 = Xp-X0 (shared across b-shifts)
                        # G_b = X0(b) + uy*D1(b) + vy*D2(b)
                        # s = G0 + ux*(Gm-G0) + vx*(Gp-G0)
                        res = {}
                        for sfx, X_t, np_ in (("A", XA, 128), ("B", XB, 16)):
                            uy = fld["uy" + sfx]
                            vy = fld["vy" + sfx]
                            D1 = lp.tile([np_, RB * Wp + 2], bf16,
                                         tag="D1" + sfx)
                            D2 = lp.tile([np_, RB * Wp + 2], bf16,
                                         tag="D2" + sfx)
                            # computed over cols [-1, N+1) relative to g
                            nc.vector.tensor_sub(
                                D1[:, 0:N + 2],
                                X_t[:, 1 * Wp - 1:1 * Wp + N + 1],
                                X_t[:, 2 * Wp - 1:2 * Wp + N + 1])
                            nc.vector.tensor_sub(
                                D2[:, 0:N + 2],
                                X_t[:, 3 * Wp - 1:3 * Wp + N + 1],
                                X_t[:, 2 * Wp - 1:2 * Wp + N + 1])
                            T = lp.tile([np_, RB * Wp], bf16, tag="T" + sfx)
                            Gs = {}
                            for bb, nm in ((-1, "Gm"), (0, "G0"), (1, "Gp")):
                                G = lp.tile([np_, RB * Wp], bf16,
                                            tag=nm + sfx)
                                X0 = X_t[:, 2 * Wp + bb:2 * Wp + bb + N]
                                nc.vector.tensor_mul(T[:, 0:N], uy[:, 0:N],
                                                     D1[:, 1 + bb:1 + bb + N])
                                nc.vector.tensor_mul(G[:, 0:N], vy[:, 0:N],
                                                     D2[:, 1 + bb:1 + bb + N])
                                nc.vector.tensor_add(G[:, 0:N], G[:, 0:N],
                                                     T[:, 0:N])
                                nc.vector.tensor_add(G[:, 0:N], G[:, 0:N], X0)
                                Gs[nm] = G
                            Gm, G0, Gp = Gs["Gm"], Gs["G0"], Gs["Gp"]
                            nc.vector.tensor_sub(Gm[:, 0:N], Gm[:, 0:N],
                                                 G0[:, 0:N])
                            nc.vector.tensor_mul(Gm[:, 0:N],
                                                 fld["ux" + sfx][:, 0:N],
                                                 Gm[:, 0:N])
                            nc.vector.tensor_sub(Gp[:, 0:N], Gp[:, 0:N],
                                                 G0[:, 0:N])
                            nc.vector.tensor_mul(Gp[:, 0:N],
                                                 fld["vx" + sfx][:, 0:N],
                                                 Gp[:, 0:N])
                            res["g" + sfx] = G0
                            res["x1" + sfx] = Gm
                            res["x2" + sfx] = Gp
                        if _dbg2 and lvl == 0 and b0 == 0:
                            for dten, stile in ((d_oy, oy72),
                                                (d_g, res["gA"]),
                                                (d_uy, fld["uyA"])):
                                cv = lp.tile([dten.shape[0], dten.shape[1]],
                                             f32, tag="dbgcv")
                                nc.vector.tensor_copy(cv[:], stile[:, 0:dten.shape[1]])
                                nc.sync.dma_start(out=dten[:], in_=cv[:])
                        # ---- einsum + bias + store ----
                        for jj in range(0, rb, 2):
                            nr = min(2, rb - jj)
                            ps = pp.tile([16, 512], f32, tag="ps16")
                            first = True
                            for sfx, wt in (("A", wtA_t), ("B", wtB_t)):
                                for t_nm in ("g", "x1", "x2"):
                                    rhs = res[t_nm + sfx][:].rearrange(
                                        "p (r c) -> p r c", c=Wp)[
                                        :, jj:jj + nr, 2:2 + Wl]
                                    nc.tensor.matmul(
                                        ps[:, 0:nr * Wl], wt[:], rhs,
                                        start=first,
                                        stop=(t_nm == "x2" and sfx == "B"))
                                    first = False
                            stg = lsp.tile([16, 2 * Wp], f32, tag="ostg")
                            nc.scalar.activation(
                                stg[:, 0:nr * Wl], ps[:, 0:nr * Wl],
                                IDENT, bias=bias_t[:, 6:7])
                            nc.sync.dma_start(
                                out=outs[lvl][:, b0 + jj:b0 + jj + nr, :],
                                in_=stg[:, 0:nr * Wl])

    nc.finalize()
    return nc


def _prep_weights(inputs, flip):
    """Host-side weight matrices for one half (flip=True for bottom half)."""
    d = {}

    def kyi(ky):
        return 2 - ky if flip else ky

    def kkmap(kk):
        ky, kx = kk // 3, kk % 3
        return ((2 - ky) * 3 + kx) if flip else kk

    for i, nm in enumerate(["fe_w1", "fe_w2", "fe_w3"]):
        w = inputs[nm]  # [16,16,3,3]
        a = np.zeros((48, 3, 16), np.float32)
        for kx in range(3):
            for ky in range(3):
                a[kx * 16:(kx + 1) * 16, ky, :] = w[:, :, kyi(ky), kx].T
        d[f"wfe{i}"] = a
    for i, nm in enumerate(["off_w0", "off_w1", "off_w2"]):
        w = inputs[nm]  # [16,32,3,3]
        a = np.zeros((96, 3, 16), np.float32)
        for kx in range(3):
            for ky in range(3):
                a[kx * 32:(kx + 1) * 32, ky, :] = w[:, :, kyi(ky), kx].T
        d[f"woff{i}"] = a
    wd = inputs["dcn_off_w"]  # [144,16,3,3]
    aY = np.zeros((48, 3, 72), np.float32)
    aX = np.zeros((48, 3, 72), np.float32)
    sgn = -1.0 if flip else 1.0
    for dg in range(DG):
        for kk in range(KK):
            ks = kkmap(kk)
            chY = (dg * KK + ks) * 2 + 0
            chX = (dg * KK + ks) * 2 + 1
            for kx in range(3):
                for ky in range(3):
                    aY[kx * 16:(kx + 1) * 16, ky, kk * 8 + dg] = \
                        sgn * wd[chY, :, kyi(ky), kx]
                    aX[kx * 16:(kx + 1) * 16, ky, kk * 8 + dg] = \
                        wd[chX, :, kyi(ky), kx]
    d["wdY"] = aY
    d["wdX"] = aX
    wdc = inputs["dcn_w"].reshape(16, 16, 9)  # [o, c, kk]
    wt = np.zeros((144, 16), np.float32)
    for dg in range(DG):
        for kk in range(KK):
            for cg in range(2):
                wt[kk * 16 + dg * 2 + cg, :] = wdc[:, dg * 2 + cg, kkmap(kk)]
    d["wtA"] = wt[:128]
    d["wtB"] = wt[128:]
    b = np.zeros((C, 8), np.float32)
    b[:, 0] = inputs["fe_b1"]
    b[:, 1] = inputs["fe_b2"]
    b[:, 2] = inputs["fe_b3"]
    b[:, 3] = inputs["off_b0"]
    b[:, 4] = inputs["off_b1"]
    b[:, 5] = inputs["off_b2"]
    b[:, 6] = inputs["dcn_b"]
    d["biases"] = b
    db = inputs["dcn_off_b"]
    b2 = np.zeros((72, 2), np.float32)
    for dg in range(DG):
        for kk in range(KK):
            ks = kkmap(kk)
            b2[kk * 8 + dg, 0] = sgn * db[(dg * KK + ks) * 2 + 0]
            b2[kk * 8 + dg, 1] = db[(dg * KK + ks) * 2 + 1]
    d["dcnb2"] = b2
    return d


def _img_band(img_b, flip):
    """img_b [16,192,192] -> [16,110,196] band, rows global [-5,105)."""
    z = np.zeros((C, 110, Wp), np.float32)
    src = img_b[:, ::-1, :] if flip else img_b
    for r in range(110):
        y = r - 5
        if 0 <= y < H:
            z[:, r, 2:194] = src[:, y, :]
    return z


def kernel(**inputs):
    from concourse.bass_utils import run_bass_kernel_spmd

    if "nc" not in _CACHE:
        _CACHE["nc"] = _build_program()
    nc = _CACHE["nc"]

    wmaps = [_prep_weights(inputs, flip) for flip in (False, True)]
    in_maps = []
    for core in range(8):
        b, s = core // 2, core % 2
        m = dict(wmaps[s])
        m["imgU"] = _img_band(np.asarray(inputs["unreg_image"][b]), s == 1)
        m["imgR"] = _img_band(np.asarray(inputs["ref_image"][b]), s == 1)
        in_maps.append(m)

    res = run_bass_kernel_spmd(nc, in_maps, list(range(8))).results

    full = [np.zeros((4, C, HI[i], WI[i]), np.float32) for i in range(3)]
    for core in range(8):
        b, s = core // 2, core % 2
        for i in range(3):
            o = res[core][f"out{i}"]  # [16, 96, WI]
            if s == 0:
                full[i][b, :, 0:RO, :] = o
            else:
                un = o[:, ::-1, :]  # orig rows [HI-96, HI)
                full[i][b, :, RO:, :] = un[:, RO - (HI[i] - RO):, :]
    return tuple(full)


# revision 20
# speedup vs baseline: 1.0142x; 1.0142x over previous
"""Trainium2 Bass kernel for nn_Algin (deformable-conv alignment).

kernel(**inputs) -> (a0, a1, a2), matching the jax reference.

Sharding: 8 cores = 4 batches x 2 H-halves (SPMD). The s=1 half runs on a
vertically FLIPPED image with ky-flipped conv weights and y-negated offset
weights, so every core executes identical top-aligned geometry (out rows
[0,96) of its flipped level); the host un-flips and assembles.

Deformable sampling uses the exact hat-function identity (valid since
|offset| < 1 here): bilinear over rows = X0 + relu(-t)*(X(-1)-X0)
+ relu(t)*(X(+1)-X0), applied separably in y then x, on a
[144=(dg,kk,cg)] partition layout with kk-pre-shifted replicas of the
zero-padded feature map. No gather needed.
"""
import numpy as np

C = 16
DG = 8
KK = 9
H = 192
Wp = 196                  # 2 + 192 + 2 padded width
WI = [192, 190, 188]      # level widths
HI = [192, 190, 188]      # level heights
RFB = [108, 106, 104]     # f1/f2/f3 band rows (row r <-> global y = r-4)
RO = 96                   # out rows computed per core per level
RB = 16                   # sampling band rows
RBF = 27                  # fe conv band rows

_CACHE = {}


def _bands(total, size):
    out, p = [], 0
    while p < total:
        out.append((p, min(size, total - p)))
        p += size
    return out


def _build_program():
    import concourse.bacc as bacc
    import concourse.mybir as mybir
    from concourse.tile import TileContext

    f32 = mybir.dt.float32
    bf16 = mybir.dt.bfloat16
    RELU = mybir.ActivationFunctionType.Relu
    IDENT = mybir.ActivationFunctionType.Identity

    nc = bacc.Bacc()

    imgU = nc.declare_dram_parameter("imgU", [C, 110, Wp], f32, isOutput=False)
    imgR = nc.declare_dram_parameter("imgR", [C, 110, Wp], f32, isOutput=False)
    wfe_d = [nc.declare_dram_parameter(f"wfe{i}", [48, 3, 16], f32, isOutput=False)
             for i in range(3)]
    woff_d = [nc.declare_dram_parameter(f"woff{i}", [96, 3, 16], f32, isOutput=False)
              for i in range(3)]
    wdY_d = nc.declare_dram_parameter("wdY", [48, 3, 72], f32, isOutput=False)
    wdX_d = nc.declare_dram_parameter("wdX", [48, 3, 72], f32, isOutput=False)
    wtA_d = nc.declare_dram_parameter("wtA", [128, 16], f32, isOutput=False)
    wtB_d = nc.declare_dram_parameter("wtB", [16, 16], f32, isOutput=False)
    bias_d = nc.declare_dram_parameter("biases", [C, 8], f32, isOutput=False)
    db2_d = nc.declare_dram_parameter("dcnb2", [72, 2], f32, isOutput=False)
    outs = [nc.declare_dram_parameter(f"out{i}", [C, RO, WI[i]], f32,
                                      isOutput=True) for i in range(3)]

    fU = [nc.dram_tensor(f"fU{i}", [C, RFB[i], Wp], bf16) for i in range(3)]
    fR = [nc.dram_tensor(f"fR{i}", [C, RFB[i], Wp], bf16) for i in range(3)]
    import os
    _dbg = os.environ.get("KDEBUG", "") == "1"
    fdbg = [nc.declare_dram_parameter(f"fdbg{i}", [C, RFB[i], Wp], f32,
                                      isOutput=True) for i in range(3)] if _dbg else None
    _dbg2 = os.environ.get("KDEBUG2", "") == "1"
    if _dbg2:
        d_oy = nc.declare_dram_parameter("d_oy", [72, RB * Wp], f32, isOutput=True)
        d_g = nc.declare_dram_parameter("d_g", [128, RB * Wp], f32, isOutput=True)
        d_uy = nc.declare_dram_parameter("d_uy", [128, RB * Wp], f32, isOutput=True)

    with TileContext(nc) as tc:
        with tc.tile_pool(name="const", bufs=1) as cpool:
            wfe_t = []
            for i in range(3):
                t = cpool.tile([48, 48], bf16, tag=f"wfe{i}")
                nc.gpsimd.dma_start(out=t[:], in_=wfe_d[i][:])
                wfe_t.append(t)
            woff_t = []
            for i in range(3):
                t = cpool.tile([96, 48], bf16, tag=f"woff{i}")
                nc.gpsimd.dma_start(out=t[:], in_=woff_d[i][:])
                woff_t.append(t)
            wdY_t = cpool.tile([48, 3 * 72], bf16, tag="wdY")
            nc.gpsimd.dma_start(out=wdY_t[:], in_=wdY_d[:])
            wdX_t = cpool.tile([48, 3 * 72], bf16, tag="wdX")
            nc.gpsimd.dma_start(out=wdX_t[:], in_=wdX_d[:])
            wtA_t = cpool.tile([128, 16], bf16, tag="wtA")
            nc.gpsimd.dma_start(out=wtA_t[:], in_=wtA_d[:])
            wtB_t = cpool.tile([16, 16], bf16, tag="wtB")
            nc.gpsimd.dma_start(out=wtB_t[:], in_=wtB_d[:])
            bias_t = cpool.tile([C, 8], f32, tag="bias")
            nc.sync.dma_start(out=bias_t[:], in_=bias_d[:])
            db2_t = cpool.tile([72, 2], f32, tag="db2")
            nc.sync.dma_start(out=db2_t[:], in_=db2_d[:])
            zerob = cpool.tile([C, 1024], bf16, tag="zerob")
            nc.vector.memset(zerob[:], 0.0)

            # ---------------- feature extraction ----------------
            with (
                tc.tile_pool(name="fe", bufs=2) as fepool,
                tc.tile_pool(name="fes", bufs=4) as fespool,
                tc.tile_pool(name="fps", bufs=4, space="PSUM") as fppool,
            ):
                def fe_conv(src, dst, wt, bcol, pad, w_out, zero_top,
                            cast=False):
                    dma_eng = nc.gpsimd if cast else nc.sync
                    rbase = 1 if pad == 1 else 2
                    base = 2
                    r_out = dst.shape[1]
                    for j0, rn in _bands(r_out, RBF):
                        patch = fepool.tile([48, (RBF + 2) * Wp], bf16,
                                            tag="fepatch")
                        pv = patch[:].rearrange("p (r c) -> p r c", c=Wp)
                        for kx in range(3):
                            dma_eng.dma_start(
                                out=pv[kx * 16:(kx + 1) * 16, 0:rn + 2,
                                       0:Wp - kx],
                                in_=src[:, j0:j0 + rn + 2, kx:Wp])
                        for jj in range(0, rn, 2):
                            nr = min(2, rn - jj)
                            ps = fppool.tile([16, 512], f32, tag="fps")
                            for ky in range(3):
                                nc.tensor.matmul(
                                    ps[:, 0:nr * w_out],
                                    wt[:, ky * 16:(ky + 1) * 16],
                                    pv[:, jj + ky:jj + ky + nr,
                                       rbase:rbase + w_out],
                                    start=(ky == 0), stop=(ky == 2))
                            stg = fespool.tile([16, 2 * Wp], bf16, tag="festg")
                            nc.scalar.activation(
                                stg[:, 0:nr * w_out], ps[:, 0:nr * w_out],
                                RELU, bias=bias_t[:, bcol:bcol + 1])
                            nc.sync.dma_start(
                                out=dst[:, j0 + jj:j0 + jj + nr,
                                        base:base + w_out],
                                in_=stg[:, 0:nr * w_out])
                    # zero horizontal pads
                    nc.sync.dma_start(out=dst[:, :, 0:base],
                                      in_=zerob[:, 0:r_out * base])
                    rw = Wp - base - w_out
                    nc.sync.dma_start(out=dst[:, :, base + w_out:Wp],
                                      in_=zerob[:, 0:r_out * rw])
                    if zero_top:
                        nc.sync.dma_start(out=dst[:, 0:4, :],
                                          in_=zerob[:, 0:4 * Wp])

                fe_conv(imgU, fU[0], wfe_t[0], 0, 1, 192, True, cast=True)
                fe_conv(fU[0], fU[1], wfe_t[1], 1, 0, 190, True)
                fe_conv(fU[1], fU[2], wfe_t[2], 2, 0, 188, True)
                if _dbg:
                    for i in range(3):
                        dt_ = fepool.tile([16, RFB[i] * Wp], f32, tag="dbg")
                        nc.sync.dma_start(out=dt_[:], in_=fU[i][:])
                        nc.sync.dma_start(out=fdbg[i][:], in_=dt_[:])
                fe_conv(imgR, fR[0], wfe_t[0], 0, 1, 192, True, cast=True)
                fe_conv(fR[0], fR[1], wfe_t[1], 1, 0, 190, True)
                fe_conv(fR[1], fR[2], wfe_t[2], 2, 0, 188, True)

            # ---------------- levels ----------------
            with (
                tc.tile_pool(name="lvl", bufs=1) as lp,
                tc.tile_pool(name="lvs", bufs=4) as lsp,
                tc.tile_pool(name="lps", bufs=4, space="PSUM") as pp,
            ):
                for lvl in range(3):
                    Wl = WI[lvl]
                    NX = (RB + 4) * Wp
                    for b0, rb in _bands(RO, RB):
                        N = rb * Wp
                        # ---- patch96 for off_feat conv ----
                        p96 = lp.tile([96, (RB + 4) * Wp], bf16, tag="p96")
                        p96v = p96[:].rearrange("p (r c) -> p r c", c=Wp)
                        for kx in range(3):
                            for im, fsrc in ((0, fR[lvl]), (1, fU[lvl])):
                                nc.sync.dma_start(
                                    out=p96v[kx * 32 + im * 16:
                                             kx * 32 + im * 16 + 16,
                                             0:rb + 4, 0:Wp - kx],
                                    in_=fsrc[:, b0 + 2:b0 + rb + 6, kx:Wp])
                        # ---- X144 replicas ----
                        XA = lp.tile([128, NX], bf16, tag="XA")
                        XB = lp.tile([16, NX], bf16, tag="XB")
                        NXb = (rb + 4) * Wp
                        sU = fU[lvl]
                        for kk in range(KK):
                            ky, kx = kk // 3, kk % 3
                            # src band row = (b0-2+rr) + (ky-1) + 4
                            r0 = b0 + ky + 1
                            c0d = max(0, 1 - kx)
                            c1d = Wp - max(0, kx - 1)
                            dstt = XA if kk < 8 else XB
                            pb = kk * 16 if kk < 8 else 0
                            nc.sync.dma_start(
                                out=dstt[:].rearrange(
                                    "p (r c) -> p r c", c=Wp)
                                [pb:pb + 16, 0:rb + 4, c0d:c1d],
                                in_=sU[:, r0:r0 + rb + 4,
                                       c0d + kx - 1:c1d + kx - 1])
                        # D1/D2 depend only on X -> emit early for overlap
                        Dts = {}
                        for sfx, X_t, np_ in (("A", XA, 128), ("B", XB, 16)):
                            D1 = lp.tile([np_, RB * Wp + 2], bf16,
                                         tag="D1" + sfx)
                            D2 = lp.tile([np_, RB * Wp + 2], bf16,
                                         tag="D2" + sfx)
                            nc.vector.tensor_sub(
                                D1[:, 0:N + 2],
                                X_t[:, 1 * Wp - 1:1 * Wp + N + 1],
                                X_t[:, 2 * Wp - 1:2 * Wp + N + 1])
                            nc.vector.tensor_sub(
                                D2[:, 0:N + 2],
                                X_t[:, 3 * Wp - 1:3 * Wp + N + 1],
                                X_t[:, 2 * Wp - 1:2 * Wp + N + 1])
                            Dts[sfx] = (D1, D2)
                        # ---- off_feat conv (pad=1, 32->16) ----
                        offt = lp.tile([16, (RB + 2) * Wp], bf16, tag="offt")
                        otv = offt[:].rearrange("p (r c) -> p r c", c=Wp)
                        for jj in range(0, rb + 2, 2):
                            nr = min(2, rb + 2 - jj)
                            ps = pp.tile([16, 512], f32, tag="ps16")
                            for ky in range(3):
                                nc.tensor.matmul(
                                    ps[:, 0:nr * Wl],
                                    woff_t[lvl][:, ky * 16:(ky + 1) * 16],
                                    p96v[:, jj + ky:jj + ky + nr, 1:1 + Wl],
                                    start=(ky == 0), stop=(ky == 2))
                            nc.scalar.activation(
                                otv[:, jj:jj + nr, 2:2 + Wl],
                                ps[:, 0:nr * Wl].rearrange(
                                    "p (r c) -> p r c", c=Wl),
                                RELU, bias=bias_t[:, 3 + lvl:4 + lvl])
                        # zero pads of off_feat (cols, and top row on band 0)
                        nc.vector.memset(otv[:, :, 0:2], 0.0)
                        nc.vector.memset(otv[:, :, 2 + Wl:Wp], 0.0)
                        if b0 == 0:
                            nc.vector.memset(otv[:, 0:1, :], 0.0)
                        # ---- dcn_off conv (pad=1, 16->72+72) ----
                        p48 = lp.tile([48, (RB + 2) * Wp], bf16, tag="p48")
                        p48v = p48[:].rearrange("p (r c) -> p r c", c=Wp)
                        for kx in range(3):
                            nc.sync.dma_start(
                                out=p48v[kx * 16:(kx + 1) * 16, :,
                                         0:Wp - kx],
                                in_=otv[:, :, kx:Wp])
                        oy72 = lp.tile([72, RB * Wp], bf16, tag="oy72")
                        ox72 = lp.tile([72, RB * Wp], bf16, tag="ox72")
                        oy72v = oy72[:].rearrange("p (r c) -> p r c", c=Wp)
                        ox72v = ox72[:].rearrange("p (r c) -> p r c", c=Wp)
                        for jj in range(0, rb, 2):
                            nr = min(2, rb - jj)
                            psY = pp.tile([72, 512], f32, tag="ps72")
                            psX = pp.tile([72, 512], f32, tag="ps72")
                            for ky in range(3):
                                nc.tensor.matmul(
                                    psY[:, 0:nr * Wl],
                                    wdY_t[:, ky * 72:(ky + 1) * 72],
                                    p48v[:, jj + ky:jj + ky + nr, 1:1 + Wl],
                                    start=(ky == 0), stop=(ky == 2))
                                nc.tensor.matmul(
                                    psX[:, 0:nr * Wl],
                                    wdX_t[:, ky * 72:(ky + 1) * 72],
                                    p48v[:, jj + ky:jj + ky + nr, 1:1 + Wl],
                                    start=(ky == 0), stop=(ky == 2))
                            nc.scalar.activation(
                                oy72v[:, jj:jj + nr, 2:2 + Wl],
                                psY[:, 0:nr * Wl].rearrange(
                                    "p (r c) -> p r c", c=Wl),
                                IDENT, bias=db2_t[:, 0:1])
                            nc.scalar.activation(
                                ox72v[:, jj:jj + nr, 2:2 + Wl],
                                psX[:, 0:nr * Wl].rearrange(
                                    "p (r c) -> p r c", c=Wl),
                                IDENT, bias=db2_t[:, 1:2])
                        # ---- dup offsets to 144 = (dg,kk,cg) ----
                        oyA = lp.tile([128, RB * Wp], bf16, tag="oyA")
                        oxA = lp.tile([128, RB * Wp], bf16, tag="oxA")
                        oyB = lp.tile([16, RB * Wp], bf16, tag="oyB")
                        oxB = lp.tile([16, RB * Wp], bf16, tag="oxB")
                        for cg in range(2):
                            nc.sync.dma_start(
                                out=oyA[:].rearrange("(u v) f -> u v f", v=2)
                                [:, cg, 0:N], in_=oy72[0:64, 0:N])
                            nc.sync.dma_start(
                                out=oxA[:].rearrange("(u v) f -> u v f", v=2)
                                [:, cg, 0:N], in_=ox72[0:64, 0:N])
                            nc.sync.dma_start(
                                out=oyB[:].rearrange("(u v) f -> u v f", v=2)
                                [:, cg, 0:N], in_=oy72[64:72, 0:N])
                            nc.sync.dma_start(
                                out=oxB[:].rearrange("(u v) f -> u v f", v=2)
                                [:, cg, 0:N], in_=ox72[64:72, 0:N])
                        # ---- hat fields ----
                        fld = {}
                        for nm, src_t, sc, npart in (
                                ("uyA", oyA, -1.0, 128), ("vyA", oyA, 1.0, 128),
                                ("uxA", oxA, -1.0, 128), ("vxA", oxA, 1.0, 128),
                                ("uyB", oyB, -1.0, 16), ("vyB", oyB, 1.0, 16),
                                ("uxB", oxB, -1.0, 16), ("vxB", oxB, 1.0, 16)):
                            t = lp.tile([npart, RB * Wp], bf16, tag=nm)
                            nc.scalar.activation(t[:, 0:N], src_t[:, 0:N],
                                                 RELU, scale=sc)
                            fld[nm] = t
                        # ---- y-lerp at 3 column shifts, then x-combine ----
                        # D1 = Xm-X0, D2 = Xp-X0 (shared across b-shifts)
                        # G_b = X0(b) + uy*D1(b) + vy*D2(b)
                        # s = G0 + ux*(Gm-G0) + vx*(Gp-G0)
                        res = {}
                        for sfx, X_t, np_ in (("A", XA, 128), ("B", XB, 16)):
                            uy = fld["uy" + sfx]
                            vy = fld["vy" + sfx]
                            D1, D2 = Dts[sfx]
                            T = lp.tile([np_, RB * Wp], bf16, tag="T" + sfx)
                            Gs = {}
                            for bb, nm in ((-1, "Gm"), (0, "G0"), (1, "Gp")):
                                G = lp.tile([np_, RB * Wp], bf16,
                                            tag=nm + sfx)
                                X0 = X_t[:, 2 * Wp + bb:2 * Wp + bb + N]
                                nc.vector.tensor_mul(T[:, 0:N], uy[:, 0:N],
                                                     D1[:, 1 + bb:1 + bb + N])
                                nc.vector.tensor_mul(G[:, 0:N], vy[:, 0:N],
                                                     D2[:, 1 + bb:1 + bb + N])
                                nc.vector.tensor_add(G[:, 0:N], G[:, 0:N],
                                                     T[:, 0:N])
                                nc.vector.tensor_add(G[:, 0:N], G[:, 0:N], X0)
                                Gs[nm] = G
                            Gm, G0, Gp = Gs["Gm"], Gs["G0"], Gs["Gp"]
                            nc.vector.tensor_sub(Gm[:, 0:N], Gm[:, 0:N],
                                                 G0[:, 0:N])
                            nc.vector.tensor_mul(Gm[:, 0:N],
                                                 fld["ux" + sfx][:, 0:N],
                                                 Gm[:, 0:N])
                            nc.vector.tensor_sub(Gp[:, 0:N], Gp[:, 0:N],
                                                 G0[:, 0:N])
                            nc.vector.tensor_mul(Gp[:, 0:N],
                                                 fld["vx" + sfx][:, 0:N],
                                                 Gp[:, 0:N])
                            res["g" + sfx] = G0
                            res["x1" + sfx] = Gm
                            res["x2" + sfx] = Gp
                        if _dbg2 and lvl == 0 and b0 == 0:
                            for dten, stile in ((d_oy, oy72),
                                                (d_g, res["gA"]),
                                                (d_uy, fld["uyA"])):
                                cv = lp.tile([dten.shape[0], dten.shape[1]],
                                             f32, tag="dbgcv")
                                nc.vector.tensor_copy(cv[:], stile[:, 0:dten.shape[1]])
                                nc.sync.dma_start(out=dten[:], in_=cv[:])
                        # ---- einsum + bias + store ----
                        for jj in range(0, rb, 2):
                            nr = min(2, rb - jj)
                            ps = pp.tile([16, 512], f32, tag="ps16")
                            first = True
                            for sfx, wt in (("A", wtA_t), ("B", wtB_t)):
                                for t_nm in ("g", "x1", "x2"):
                                    rhs = res[t_nm + sfx][:].rearrange(
                                        "p (r c) -> p r c", c=Wp)[
                                        :, jj:jj + nr, 2:2 + Wl]
                                    nc.tensor.matmul(
                                        ps[:, 0:nr * Wl], wt[:], rhs,
                                        start=first,
                                        stop=(t_nm == "x2" and sfx == "B"))
                                    first = False
                            stg = lsp.tile([16, 2 * Wp], f32, tag="ostg")
                            nc.scalar.activation(
                                stg[:, 0:nr * Wl], ps[:, 0:nr * Wl],
                                IDENT, bias=bias_t[:, 6:7])
                            nc.sync.dma_start(
                                out=outs[lvl][:, b0 + jj:b0 + jj + nr, :],
                                in_=stg[:, 0:nr * Wl])

    nc.finalize()
    return nc


def _prep_weights(inputs, flip):
    """Host-side weight matrices for one half (flip=True for bottom half)."""
    d = {}

    def kyi(ky):
        return 2 - ky if flip else ky

    def kkmap(kk):
        ky, kx = kk // 3, kk % 3
        return ((2 - ky) * 3 + kx) if flip else kk

    for i, nm in enumerate(["fe_w1", "fe_w2", "fe_w3"]):
        w = inputs[nm]  # [16,16,3,3]
        a = np.zeros((48, 3, 16), np.float32)
        for kx in range(3):
            for ky in range(3):
                a[kx * 16:(kx + 1) * 16, ky, :] = w[:, :, kyi(ky), kx].T
        d[f"wfe{i}"] = a
    for i, nm in enumerate(["off_w0", "off_w1", "off_w2"]):
        w = inputs[nm]  # [16,32,3,3]
        a = np.zeros((96, 3, 16), np.float32)
        for kx in range(3):
            for ky in range(3):
                a[kx * 32:(kx + 1) * 32, ky, :] = w[:, :, kyi(ky), kx].T
        d[f"woff{i}"] = a
    wd = inputs["dcn_off_w"]  # [144,16,3,3]
    aY = np.zeros((48, 3, 72), np.float32)
    aX = np.zeros((48, 3, 72), np.float32)
    sgn = -1.0 if flip else 1.0
    for dg in range(DG):
        for kk in range(KK):
            ks = kkmap(kk)
            chY = (dg * KK + ks) * 2 + 0
            chX = (dg * KK + ks) * 2 + 1
            for kx in range(3):
                for ky in range(3):
                    aY[kx * 16:(kx + 1) * 16, ky, kk * 8 + dg] = \
                        sgn * wd[chY, :, kyi(ky), kx]
                    aX[kx * 16:(kx + 1) * 16, ky, kk * 8 + dg] = \
                        wd[chX, :, kyi(ky), kx]
    d["wdY"] = aY
    d["wdX"] = aX
    wdc = inputs["dcn_w"].reshape(16, 16, 9)  # [o, c, kk]
    wt = np.zeros((144, 16), np.float32)
    for dg in range(DG):
        for kk in range(KK):
            for cg in range(2):
                wt[kk * 16 + dg * 2 + cg, :] = wdc[:, dg * 2 + cg, kkmap(kk)]
    d["wtA"] = wt[:128]
    d["wtB"] = wt[128:]
    b = np.zeros((C, 8), np.float32)
    b[:, 0] = inputs["fe_b1"]
    b[:, 1] = inputs["fe_b2"]
    b[:, 2] = inputs["fe_b3"]
    b[:, 3] = inputs["off_b0"]
    b[:, 4] = inputs["off_b1"]
    b[:, 5] = inputs["off_b2"]
    b[:, 6] = inputs["dcn_b"]
    d["biases"] = b
    db = inputs["dcn_off_b"]
    b2 = np.zeros((72, 2), np.float32)
    for dg in range(DG):
        for kk in range(KK):
            ks = kkmap(kk)
            b2[kk * 8 + dg, 0] = sgn * db[(dg * KK + ks) * 2 + 0]
            b2[kk * 8 + dg, 1] = db[(dg * KK + ks) * 2 + 1]
    d["dcnb2"] = b2
    return d


def _img_band(img_b, flip):
    """img_b [16,192,192] -> [16,110,196] band, rows global [-5,105)."""
    z = np.zeros((C, 110, Wp), np.float32)
    src = img_b[:, ::-1, :] if flip else img_b
    for r in range(110):
        y = r - 5
        if 0 <= y < H:
            z[:, r, 2:194] = src[:, y, :]
    return z


def kernel(**inputs):
    from concourse.bass_utils import run_bass_kernel_spmd

    if "nc" not in _CACHE:
        _CACHE["nc"] = _build_program()
    nc = _CACHE["nc"]

    wmaps = [_prep_weights(inputs, flip) for flip in (False, True)]
    in_maps = []
    for core in range(8):
        b, s = core // 2, core % 2
        m = dict(wmaps[s])
        m["imgU"] = _img_band(np.asarray(inputs["unreg_image"][b]), s == 1)
        m["imgR"] = _img_band(np.asarray(inputs["ref_image"][b]), s == 1)
        in_maps.append(m)

    res = run_bass_kernel_spmd(nc, in_maps, list(range(8))).results

    full = [np.zeros((4, C, HI[i], WI[i]), np.float32) for i in range(3)]
    for core in range(8):
        b, s = core // 2, core % 2
        for i in range(3):
            o = res[core][f"out{i}"]  # [16, 96, WI]
            if s == 0:
                full[i][b, :, 0:RO, :] = o
            else:
                un = o[:, ::-1, :]  # orig rows [HI-96, HI)
                full[i][b, :, RO:, :] = un[:, RO - (HI[i] - RO):, :]
    return tuple(full)


# revision 21
# speedup vs baseline: 1.1453x; 1.1292x over previous
"""Trainium2 Bass kernel for nn_Algin (deformable-conv alignment).

kernel(**inputs) -> (a0, a1, a2), matching the jax reference.

Sharding: 8 cores = 4 batches x 2 H-halves (SPMD). The s=1 half runs on a
vertically FLIPPED image with ky-flipped conv weights and y-negated offset
weights, so every core executes identical top-aligned geometry (out rows
[0,96) of its flipped level); the host un-flips and assembles.

Deformable sampling uses the exact hat-function identity (valid since
|offset| < 1 here): bilinear over rows = X0 + relu(-t)*(X(-1)-X0)
+ relu(t)*(X(+1)-X0), applied separably in y then x, on a
[144=(dg,kk,cg)] partition layout with kk-pre-shifted replicas of the
zero-padded feature map. No gather needed.
"""
import numpy as np

C = 16
DG = 8
KK = 9
H = 192
Wp = 196                  # 2 + 192 + 2 padded width
WI = [192, 190, 188]      # level widths
HI = [192, 190, 188]      # level heights
RFB = [108, 106, 104]     # f1/f2/f3 band rows (row r <-> global y = r-4)
RO = 96                   # out rows computed per core per level
RB = 16                   # sampling band rows
RBF = 54                  # fe conv band rows

_CACHE = {}


def _bands(total, size):
    out, p = [], 0
    while p < total:
        out.append((p, min(size, total - p)))
        p += size
    return out


def _build_program():
    import concourse.bacc as bacc
    import concourse.mybir as mybir
    from concourse.tile import TileContext

    f32 = mybir.dt.float32
    bf16 = mybir.dt.bfloat16
    RELU = mybir.ActivationFunctionType.Relu
    IDENT = mybir.ActivationFunctionType.Identity

    nc = bacc.Bacc()

    imgU = nc.declare_dram_parameter("imgU", [C, 110, Wp], f32, isOutput=False)
    imgR = nc.declare_dram_parameter("imgR", [C, 110, Wp], f32, isOutput=False)
    wfe_d = [nc.declare_dram_parameter(f"wfe{i}", [48, 3, 16], f32, isOutput=False)
             for i in range(3)]
    woff_d = [nc.declare_dram_parameter(f"woff{i}", [96, 3, 16], f32, isOutput=False)
              for i in range(3)]
    wdY_d = nc.declare_dram_parameter("wdY", [48, 3, 72], f32, isOutput=False)
    wdX_d = nc.declare_dram_parameter("wdX", [48, 3, 72], f32, isOutput=False)
    wtA_d = nc.declare_dram_parameter("wtA", [128, 16], f32, isOutput=False)
    wtB_d = nc.declare_dram_parameter("wtB", [16, 16], f32, isOutput=False)
    bias_d = nc.declare_dram_parameter("biases", [C, 8], f32, isOutput=False)
    db2_d = nc.declare_dram_parameter("dcnb2", [72, 2], f32, isOutput=False)
    outs = [nc.declare_dram_parameter(f"out{i}", [C, RO, WI[i]], f32,
                                      isOutput=True) for i in range(3)]

    fU = [nc.dram_tensor(f"fU{i}", [C, RFB[i], Wp], bf16) for i in range(3)]
    fR = [nc.dram_tensor(f"fR{i}", [C, RFB[i], Wp], bf16) for i in range(3)]
    import os
    _dbg = os.environ.get("KDEBUG", "") == "1"
    fdbg = [nc.declare_dram_parameter(f"fdbg{i}", [C, RFB[i], Wp], f32,
                                      isOutput=True) for i in range(3)] if _dbg else None
    _dbg2 = os.environ.get("KDEBUG2", "") == "1"
    if _dbg2:
        d_oy = nc.declare_dram_parameter("d_oy", [72, RB * Wp], f32, isOutput=True)
        d_g = nc.declare_dram_parameter("d_g", [128, RB * Wp], f32, isOutput=True)
        d_uy = nc.declare_dram_parameter("d_uy", [128, RB * Wp], f32, isOutput=True)

    with TileContext(nc) as tc:
        with tc.tile_pool(name="const", bufs=1) as cpool:
            wfe_t = []
            for i in range(3):
                t = cpool.tile([48, 48], bf16, tag=f"wfe{i}")
                nc.gpsimd.dma_start(out=t[:], in_=wfe_d[i][:])
                wfe_t.append(t)
            woff_t = []
            for i in range(3):
                t = cpool.tile([96, 48], bf16, tag=f"woff{i}")
                nc.gpsimd.dma_start(out=t[:], in_=woff_d[i][:])
                woff_t.append(t)
            wdY_t = cpool.tile([48, 3 * 72], bf16, tag="wdY")
            nc.gpsimd.dma_start(out=wdY_t[:], in_=wdY_d[:])
            wdX_t = cpool.tile([48, 3 * 72], bf16, tag="wdX")
            nc.gpsimd.dma_start(out=wdX_t[:], in_=wdX_d[:])
            wtA_t = cpool.tile([128, 16], bf16, tag="wtA")
            nc.gpsimd.dma_start(out=wtA_t[:], in_=wtA_d[:])
            wtB_t = cpool.tile([16, 16], bf16, tag="wtB")
            nc.gpsimd.dma_start(out=wtB_t[:], in_=wtB_d[:])
            bias_t = cpool.tile([C, 8], f32, tag="bias")
            nc.sync.dma_start(out=bias_t[:], in_=bias_d[:])
            db2_t = cpool.tile([72, 2], f32, tag="db2")
            nc.sync.dma_start(out=db2_t[:], in_=db2_d[:])
            zerob = cpool.tile([C, 1024], bf16, tag="zerob")
            nc.vector.memset(zerob[:], 0.0)

            # ---------------- feature extraction ----------------
            with (
                tc.tile_pool(name="fe", bufs=2) as fepool,
                tc.tile_pool(name="fes", bufs=4) as fespool,
                tc.tile_pool(name="fps", bufs=4, space="PSUM") as fppool,
            ):
                def fe_conv(src, dst, wt, bcol, pad, w_out, zero_top,
                            cast=False):
                    dma_eng = nc.gpsimd if cast else nc.sync
                    rbase = 1 if pad == 1 else 2
                    base = 2
                    r_out = dst.shape[1]
                    for j0, rn in _bands(r_out, RBF):
                        patch = fepool.tile([48, (RBF + 2) * Wp], bf16,
                                            tag="fepatch")
                        pv = patch[:].rearrange("p (r c) -> p r c", c=Wp)
                        for kx in range(3):
                            dma_eng.dma_start(
                                out=pv[kx * 16:(kx + 1) * 16, 0:rn + 2,
                                       0:Wp - kx],
                                in_=src[:, j0:j0 + rn + 2, kx:Wp])
                        for jj in range(0, rn, 2):
                            nr = min(2, rn - jj)
                            ps = fppool.tile([16, 512], f32, tag="fps")
                            for ky in range(3):
                                nc.tensor.matmul(
                                    ps[:, 0:nr * w_out],
                                    wt[:, ky * 16:(ky + 1) * 16],
                                    pv[:, jj + ky:jj + ky + nr,
                                       rbase:rbase + w_out],
                                    start=(ky == 0), stop=(ky == 2))
                            stg = fespool.tile([16, 2 * Wp], bf16, tag="festg")
                            nc.scalar.activation(
                                stg[:, 0:nr * w_out], ps[:, 0:nr * w_out],
                                RELU, bias=bias_t[:, bcol:bcol + 1])
                            nc.sync.dma_start(
                                out=dst[:, j0 + jj:j0 + jj + nr,
                                        base:base + w_out],
                                in_=stg[:, 0:nr * w_out])
                    # zero horizontal pads
                    nc.sync.dma_start(out=dst[:, :, 0:base],
                                      in_=zerob[:, 0:r_out * base])
                    rw = Wp - base - w_out
                    nc.sync.dma_start(out=dst[:, :, base + w_out:Wp],
                                      in_=zerob[:, 0:r_out * rw])
                    if zero_top:
                        nc.sync.dma_start(out=dst[:, 0:4, :],
                                          in_=zerob[:, 0:4 * Wp])

                fe_conv(imgU, fU[0], wfe_t[0], 0, 1, 192, True, cast=True)
                fe_conv(fU[0], fU[1], wfe_t[1], 1, 0, 190, True)
                fe_conv(fU[1], fU[2], wfe_t[2], 2, 0, 188, True)
                if _dbg:
                    for i in range(3):
                        dt_ = fepool.tile([16, RFB[i] * Wp], f32, tag="dbg")
                        nc.sync.dma_start(out=dt_[:], in_=fU[i][:])
                        nc.sync.dma_start(out=fdbg[i][:], in_=dt_[:])
                fe_conv(imgR, fR[0], wfe_t[0], 0, 1, 192, True, cast=True)
                fe_conv(fR[0], fR[1], wfe_t[1], 1, 0, 190, True)
                fe_conv(fR[1], fR[2], wfe_t[2], 2, 0, 188, True)

            # ---------------- levels ----------------
            with (
                tc.tile_pool(name="lvl", bufs=1) as lp,
                tc.tile_pool(name="lvs", bufs=4) as lsp,
                tc.tile_pool(name="lps", bufs=4, space="PSUM") as pp,
            ):
                for lvl in range(3):
                    Wl = WI[lvl]
                    NX = (RB + 4) * Wp
                    for b0, rb in _bands(RO, RB):
                        N = rb * Wp
                        # ---- patch96 for off_feat conv ----
                        p96 = lp.tile([96, (RB + 4) * Wp], bf16, tag="p96")
                        p96v = p96[:].rearrange("p (r c) -> p r c", c=Wp)
                        for kx in range(3):
                            for im, fsrc in ((0, fR[lvl]), (1, fU[lvl])):
                                nc.sync.dma_start(
                                    out=p96v[kx * 32 + im * 16:
                                             kx * 32 + im * 16 + 16,
                                             0:rb + 4, 0:Wp - kx],
                                    in_=fsrc[:, b0 + 2:b0 + rb + 6, kx:Wp])
                        # ---- X144 replicas ----
                        XA = lp.tile([128, NX], bf16, tag="XA")
                        XB = lp.tile([16, NX], bf16, tag="XB")
                        NXb = (rb + 4) * Wp
                        sU = fU[lvl]
                        for kk in range(KK):
                            ky, kx = kk // 3, kk % 3
                            # src band row = (b0-2+rr) + (ky-1) + 4
                            r0 = b0 + ky + 1
                            c0d = max(0, 1 - kx)
                            c1d = Wp - max(0, kx - 1)
                            dstt = XA if kk < 8 else XB
                            pb = kk * 16 if kk < 8 else 0
                            nc.sync.dma_start(
                                out=dstt[:].rearrange(
                                    "p (r c) -> p r c", c=Wp)
                                [pb:pb + 16, 0:rb + 4, c0d:c1d],
                                in_=sU[:, r0:r0 + rb + 4,
                                       c0d + kx - 1:c1d + kx - 1])
                        # D1/D2 depend only on X -> emit early for overlap
                        Dts = {}
                        for sfx, X_t, np_ in (("A", XA, 128), ("B", XB, 16)):
                            D1 = lp.tile([np_, RB * Wp + 2], bf16,
                                         tag="D1" + sfx)
                            D2 = lp.tile([np_, RB * Wp + 2], bf16,
                                         tag="D2" + sfx)
                            nc.vector.tensor_sub(
                                D1[:, 0:N + 2],
                                X_t[:, 1 * Wp - 1:1 * Wp + N + 1],
                                X_t[:, 2 * Wp - 1:2 * Wp + N + 1])
                            nc.vector.tensor_sub(
                                D2[:, 0:N + 2],
                                X_t[:, 3 * Wp - 1:3 * Wp + N + 1],
                                X_t[:, 2 * Wp - 1:2 * Wp + N + 1])
                            Dts[sfx] = (D1, D2)
                        # ---- off_feat conv (pad=1, 32->16) ----
                        offt = lp.tile([16, (RB + 2) * Wp], bf16, tag="offt")
                        otv = offt[:].rearrange("p (r c) -> p r c", c=Wp)
                        for jj in range(0, rb + 2, 2):
                            nr = min(2, rb + 2 - jj)
                            ps = pp.tile([16, 512], f32, tag="ps16")
                            for ky in range(3):
                                nc.tensor.matmul(
                                    ps[:, 0:nr * Wl],
                                    woff_t[lvl][:, ky * 16:(ky + 1) * 16],
                                    p96v[:, jj + ky:jj + ky + nr, 1:1 + Wl],
                                    start=(ky == 0), stop=(ky == 2))
                            nc.scalar.activation(
                                otv[:, jj:jj + nr, 2:2 + Wl],
                                ps[:, 0:nr * Wl].rearrange(
                                    "p (r c) -> p r c", c=Wl),
                                RELU, bias=bias_t[:, 3 + lvl:4 + lvl])
                        # zero pads of off_feat (cols, and top row on band 0)
                        nc.vector.memset(otv[:, :, 0:2], 0.0)
                        nc.vector.memset(otv[:, :, 2 + Wl:Wp], 0.0)
                        if b0 == 0:
                            nc.vector.memset(otv[:, 0:1, :], 0.0)
                        # ---- dcn_off conv (pad=1, 16->72+72) ----
                        p48 = lp.tile([48, (RB + 2) * Wp], bf16, tag="p48")
                        p48v = p48[:].rearrange("p (r c) -> p r c", c=Wp)
                        for kx in range(3):
                            nc.sync.dma_start(
                                out=p48v[kx * 16:(kx + 1) * 16, :,
                                         0:Wp - kx],
                                in_=otv[:, :, kx:Wp])
                        oy72 = lp.tile([72, RB * Wp], bf16, tag="oy72")
                        ox72 = lp.tile([72, RB * Wp], bf16, tag="ox72")
                        oy72v = oy72[:].rearrange("p (r c) -> p r c", c=Wp)
                        ox72v = ox72[:].rearrange("p (r c) -> p r c", c=Wp)
                        for jj in range(0, rb, 2):
                            nr = min(2, rb - jj)
                            psY = pp.tile([72, 512], f32, tag="ps72")
                            psX = pp.tile([72, 512], f32, tag="ps72")
                            for ky in range(3):
                                nc.tensor.matmul(
                                    psY[:, 0:nr * Wl],
                                    wdY_t[:, ky * 72:(ky + 1) * 72],
                                    p48v[:, jj + ky:jj + ky + nr, 1:1 + Wl],
                                    start=(ky == 0), stop=(ky == 2))
                                nc.tensor.matmul(
                                    psX[:, 0:nr * Wl],
                                    wdX_t[:, ky * 72:(ky + 1) * 72],
                                    p48v[:, jj + ky:jj + ky + nr, 1:1 + Wl],
                                    start=(ky == 0), stop=(ky == 2))
                            nc.scalar.activation(
                                oy72v[:, jj:jj + nr, 2:2 + Wl],
                                psY[:, 0:nr * Wl].rearrange(
                                    "p (r c) -> p r c", c=Wl),
                                IDENT, bias=db2_t[:, 0:1])
                            nc.scalar.activation(
                                ox72v[:, jj:jj + nr, 2:2 + Wl],
                                psX[:, 0:nr * Wl].rearrange(
                                    "p (r c) -> p r c", c=Wl),
                                IDENT, bias=db2_t[:, 1:2])
                        # ---- dup offsets to 144 = (dg,kk,cg) ----
                        oyA = lp.tile([128, RB * Wp], bf16, tag="oyA")
                        oxA = lp.tile([128, RB * Wp], bf16, tag="oxA")
                        oyB = lp.tile([16, RB * Wp], bf16, tag="oyB")
                        oxB = lp.tile([16, RB * Wp], bf16, tag="oxB")
                        for cg in range(2):
                            nc.sync.dma_start(
                                out=oyA[:].rearrange("(u v) f -> u v f", v=2)
                                [:, cg, 0:N], in_=oy72[0:64, 0:N])
                            nc.sync.dma_start(
                                out=oxA[:].rearrange("(u v) f -> u v f", v=2)
                                [:, cg, 0:N], in_=ox72[0:64, 0:N])
                            nc.sync.dma_start(
                                out=oyB[:].rearrange("(u v) f -> u v f", v=2)
                                [:, cg, 0:N], in_=oy72[64:72, 0:N])
                            nc.sync.dma_start(
                                out=oxB[:].rearrange("(u v) f -> u v f", v=2)
                                [:, cg, 0:N], in_=ox72[64:72, 0:N])
                        # ---- hat fields ----
                        fld = {}
                        for nm, src_t, sc, npart in (
                                ("uyA", oyA, -1.0, 128), ("vyA", oyA, 1.0, 128),
                                ("uxA", oxA, -1.0, 128), ("vxA", oxA, 1.0, 128),
                                ("uyB", oyB, -1.0, 16), ("vyB", oyB, 1.0, 16),
                                ("uxB", oxB, -1.0, 16), ("vxB", oxB, 1.0, 16)):
                            t = lp.tile([npart, RB * Wp], bf16, tag=nm)
                            nc.scalar.activation(t[:, 0:N], src_t[:, 0:N],
                                                 RELU, scale=sc)
                            fld[nm] = t
                        # ---- y-lerp at 3 column shifts, then x-combine ----
                        # D1 = Xm-X0, D2 = Xp-X0 (shared across b-shifts)
                        # G_b = X0(b) + uy*D1(b) + vy*D2(b)
                        # s = G0 + ux*(Gm-G0) + vx*(Gp-G0)
                        res = {}
                        for sfx, X_t, np_ in (("A", XA, 128), ("B", XB, 16)):
                            uy = fld["uy" + sfx]
                            vy = fld["vy" + sfx]
                            D1, D2 = Dts[sfx]
                            T = lp.tile([np_, RB * Wp], bf16, tag="T" + sfx)
                            Gs = {}
                            for bb, nm in ((-1, "Gm"), (0, "G0"), (1, "Gp")):
                                G = lp.tile([np_, RB * Wp], bf16,
                                            tag=nm + sfx)
                                X0 = X_t[:, 2 * Wp + bb:2 * Wp + bb + N]
                                nc.vector.tensor_mul(T[:, 0:N], uy[:, 0:N],
                                                     D1[:, 1 + bb:1 + bb + N])
                                nc.vector.tensor_mul(G[:, 0:N], vy[:, 0:N],
                                                     D2[:, 1 + bb:1 + bb + N])
                                nc.vector.tensor_add(G[:, 0:N], G[:, 0:N],
                                                     T[:, 0:N])
                                nc.vector.tensor_add(G[:, 0:N], G[:, 0:N], X0)
                                Gs[nm] = G
                            Gm, G0, Gp = Gs["Gm"], Gs["G0"], Gs["Gp"]
                            nc.vector.tensor_sub(Gm[:, 0:N], Gm[:, 0:N],
                                                 G0[:, 0:N])
                            nc.vector.tensor_mul(Gm[:, 0:N],
                                                 fld["ux" + sfx][:, 0:N],
                                                 Gm[:, 0:N])
                            nc.vector.tensor_sub(Gp[:, 0:N], Gp[:, 0:N],
                                                 G0[:, 0:N])
                            nc.vector.tensor_mul(Gp[:, 0:N],
                                                 fld["vx" + sfx][:, 0:N],
                                                 Gp[:, 0:N])
                            res["g" + sfx] = G0
                            res["x1" + sfx] = Gm
                            res["x2" + sfx] = Gp
                        if _dbg2 and lvl == 0 and b0 == 0:
                            for dten, stile in ((d_oy, oy72),
                                                (d_g, res["gA"]),
                                                (d_uy, fld["uyA"])):
                                cv = lp.tile([dten.shape[0], dten.shape[1]],
                                             f32, tag="dbgcv")
                                nc.vector.tensor_copy(cv[:], stile[:, 0:dten.shape[1]])
                                nc.sync.dma_start(out=dten[:], in_=cv[:])
                        # ---- einsum + bias + store ----
                        for jj in range(0, rb, 2):
                            nr = min(2, rb - jj)
                            ps = pp.tile([16, 512], f32, tag="ps16")
                            first = True
                            for sfx, wt in (("A", wtA_t), ("B", wtB_t)):
                                for t_nm in ("g", "x1", "x2"):
                                    rhs = res[t_nm + sfx][:].rearrange(
                                        "p (r c) -> p r c", c=Wp)[
                                        :, jj:jj + nr, 2:2 + Wl]
                                    nc.tensor.matmul(
                                        ps[:, 0:nr * Wl], wt[:], rhs,
                                        start=first,
                                        stop=(t_nm == "x2" and sfx == "B"))
                                    first = False
                            stg = lsp.tile([16, 2 * Wp], f32, tag="ostg")
                            nc.scalar.activation(
                                stg[:, 0:nr * Wl], ps[:, 0:nr * Wl],
                                IDENT, bias=bias_t[:, 6:7])
                            nc.sync.dma_start(
                                out=outs[lvl][:, b0 + jj:b0 + jj + nr, :],
                                in_=stg[:, 0:nr * Wl])

    nc.finalize()
    return nc


def _prep_weights(inputs, flip):
    """Host-side weight matrices for one half (flip=True for bottom half)."""
    d = {}

    def kyi(ky):
        return 2 - ky if flip else ky

    def kkmap(kk):
        ky, kx = kk // 3, kk % 3
        return ((2 - ky) * 3 + kx) if flip else kk

    for i, nm in enumerate(["fe_w1", "fe_w2", "fe_w3"]):
        w = inputs[nm]  # [16,16,3,3]
        a = np.zeros((48, 3, 16), np.float32)
        for kx in range(3):
            for ky in range(3):
                a[kx * 16:(kx + 1) * 16, ky, :] = w[:, :, kyi(ky), kx].T
        d[f"wfe{i}"] = a
    for i, nm in enumerate(["off_w0", "off_w1", "off_w2"]):
        w = inputs[nm]  # [16,32,3,3]
        a = np.zeros((96, 3, 16), np.float32)
        for kx in range(3):
            for ky in range(3):
                a[kx * 32:(kx + 1) * 32, ky, :] = w[:, :, kyi(ky), kx].T
        d[f"woff{i}"] = a
    wd = inputs["dcn_off_w"]  # [144,16,3,3]
    aY = np.zeros((48, 3, 72), np.float32)
    aX = np.zeros((48, 3, 72), np.float32)
    sgn = -1.0 if flip else 1.0
    for dg in range(DG):
        for kk in range(KK):
            ks = kkmap(kk)
            chY = (dg * KK + ks) * 2 + 0
            chX = (dg * KK + ks) * 2 + 1
            for kx in range(3):
                for ky in range(3):
                    aY[kx * 16:(kx + 1) * 16, ky, kk * 8 + dg] = \
                        sgn * wd[chY, :, kyi(ky), kx]
                    aX[kx * 16:(kx + 1) * 16, ky, kk * 8 + dg] = \
                        wd[chX, :, kyi(ky), kx]
    d["wdY"] = aY
    d["wdX"] = aX
    wdc = inputs["dcn_w"].reshape(16, 16, 9)  # [o, c, kk]
    wt = np.zeros((144, 16), np.float32)
    for dg in range(DG):
        for kk in range(KK):
            for cg in range(2):
                wt[kk * 16 + dg * 2 + cg, :] = wdc[:, dg * 2 + cg, kkmap(kk)]
    d["wtA"] = wt[:128]
    d["wtB"] = wt[128:]
    b = np.zeros((C, 8), np.float32)
    b[:, 0] = inputs["fe_b1"]
    b[:, 1] = inputs["fe_b2"]
    b[:, 2] = inputs["fe_b3"]
    b[:, 3] = inputs["off_b0"]
    b[:, 4] = inputs["off_b1"]
    b[:, 5] = inputs["off_b2"]
    b[:, 6] = inputs["dcn_b"]
    d["biases"] = b
    db = inputs["dcn_off_b"]
    b2 = np.zeros((72, 2), np.float32)
    for dg in range(DG):
        for kk in range(KK):
            ks = kkmap(kk)
            b2[kk * 8 + dg, 0] = sgn * db[(dg * KK + ks) * 2 + 0]
            b2[kk * 8 + dg, 1] = db[(dg * KK + ks) * 2 + 1]
    d["dcnb2"] = b2
    return d


def _img_band(img_b, flip):
    """img_b [16,192,192] -> [16,110,196] band, rows global [-5,105)."""
    z = np.zeros((C, 110, Wp), np.float32)
    src = img_b[:, ::-1, :] if flip else img_b
    for r in range(110):
        y = r - 5
        if 0 <= y < H:
            z[:, r, 2:194] = src[:, y, :]
    return z


def kernel(**inputs):
    from concourse.bass_utils import run_bass_kernel_spmd

    if "nc" not in _CACHE:
        _CACHE["nc"] = _build_program()
    nc = _CACHE["nc"]

    wmaps = [_prep_weights(inputs, flip) for flip in (False, True)]
    in_maps = []
    for core in range(8):
        b, s = core // 2, core % 2
        m = dict(wmaps[s])
        m["imgU"] = _img_band(np.asarray(inputs["unreg_image"][b]), s == 1)
        m["imgR"] = _img_band(np.asarray(inputs["ref_image"][b]), s == 1)
        in_maps.append(m)

    res = run_bass_kernel_spmd(nc, in_maps, list(range(8))).results

    full = [np.zeros((4, C, HI[i], WI[i]), np.float32) for i in range(3)]
    for core in range(8):
        b, s = core // 2, core % 2
        for i in range(3):
            o = res[core][f"out{i}"]  # [16, 96, WI]
            if s == 0:
                full[i][b, :, 0:RO, :] = o
            else:
                un = o[:, ::-1, :]  # orig rows [HI-96, HI)
                full[i][b, :, RO:, :] = un[:, RO - (HI[i] - RO):, :]
    return tuple(full)
